# revision 15
# baseline (speedup 1.0000x reference)
"""Trainium2 Bass kernel for nn_Block_8031588843661 (dense_transformer).

The reference block is pointwise over (b, t) tokens: the "attention" runs over
the 4 heads *within* each token (4x4 scores, causal over the head index), so a
pure data-parallel shard of B=4096 across 8 cores needs no communication.

Per-token math (token x of 512 = 4 head-chunks xh_j of 128):
    l[i,j] = xh_i . (G xh_j),  G = Wq Wk^T / sqrt(512)     (j <= i, softmax rows)
    y[i,:] = sum_j wei[i,j] * u_j + bp,  u_j = xh_j @ (Wv Wp)
    x1     = LN_eps512(x + y)
    h      = gelu(x1 @ W1 + b1);  h2 = h @ W2 + b2
    out    = LN_eps512(x1 + h2)                            -> [4, 4, 512] per token

Folding Wq Wk^T and Wv Wp on the host removes the q/k/proj matmuls entirely.
On-chip layout: token-major [128 tokens x feat] for scores/softmax/mix/LN,
feature-major (PE-transposed) for the FFN matmuls, which run in float32r
(fp32 with 11-bit mantissa, full PE rate at free-dim >= 256).

Per core: 512 batches = 2048 tokens = 16 tiles of 128 tokens.
"""

import numpy as np

import concourse.bass as bass
import concourse.bacc as bacc
import concourse.tile as tile
import concourse.mybir as mybir
from concourse import bass_utils
from concourse.masks import make_identity

F32 = mybir.dt.float32
F32R = mybir.dt.float32r
AF = mybir.ActivationFunctionType
OP = mybir.AluOpType

B, T, E, H, DH, FF = 4096, 4, 512, 4, 128, 2048
NCORES = 8
BS = B // NCORES              # 512 batches per core
TOK = BS * T                  # 2048 tokens per core
NT = TOK // 128               # 16 token tiles
LN_EPS = 512.0
RSQRT_Y0 = float((512.0 + 1.5) ** -0.5)   # var+eps lands in ~[512, 515]

# causal (i, j<=i) score pairs for rows i >= 1, in logits-column order
PAIRS = [(1, 0), (1, 1), (2, 0), (2, 1), (2, 2), (3, 0), (3, 1), (3, 2), (3, 3)]
ROW_COLS = {1: (0, 2), 2: (2, 5), 3: (5, 9)}   # logit column ranges per row


def _round_f32r(x: np.ndarray) -> np.ndarray:
    """Round fp32 to fp32r (RNE to 11-bit mantissa) — matches HW cast."""
    u = np.ascontiguousarray(x, dtype=np.float32).view(np.uint32)
    r = (u + (0x7FF + ((u >> 12) & 1))) & np.uint32(0xFFFFF000)
    return r.view(np.float32)


def _rsqrt_dve(nc, pool, a, n):
    """rstd = 1/sqrt(a) on DVE only (no ACT table), a ~ 512..600.
    a: [128, n, 1] fp32 AP. Returns [128, n, 1] tile."""
    y = pool.tile([128, n, 1], F32, tag="rs_y")
    t = pool.tile([128, n, 1], F32, tag="rs_t")
    nc.vector.tensor_scalar(out=y, in0=a, scalar1=0.0, scalar2=RSQRT_Y0,
                            op0=OP.mult, op1=OP.add)
    for _ in range(3):
        nc.vector.tensor_mul(out=t, in0=y, in1=y)
        nc.vector.tensor_mul(out=t, in0=t, in1=a)
        nc.vector.tensor_scalar(out=t, in0=t, scalar1=-0.5, scalar2=1.5,
                                op0=OP.mult, op1=OP.add)
        nc.vector.tensor_mul(out=y, in0=y, in1=t)
    return y


def _layernorm(nc, pool, s, out, tag):
    """out = LN_eps512(s) per 512-feat segment. s/out: [128, 4, 512] SBUF tiles."""
    stats = pool.tile([128, 4, 6], F32, tag=f"{tag}_st")
    mv = pool.tile([128, 4, 2], F32, tag=f"{tag}_mv")
    for i in range(4):
        nc.vector.bn_stats(out=stats[:, i, :], in_=s[:, i, :])
        nc.vector.bn_aggr(out=mv[:, i, :], in_=stats[:, i, :])
    a = pool.tile([128, 4, 1], F32, tag=f"{tag}_a")      # var + eps
    nc.vector.tensor_scalar(out=a, in0=mv[:, :, 1:2], scalar1=LN_EPS, scalar2=None,
                            op0=OP.add)
    rstd = _rsqrt_dve(nc, pool, a, 4)
    for i in range(4):
        nc.vector.tensor_scalar(out=out[:, i, :], in0=s[:, i, :],
                                scalar1=mv[:, i, 0:1], scalar2=rstd[:, i, :],
                                op0=OP.subtract, op1=OP.mult)


def build_bass(debug=False):
    nc = bacc.Bacc("TRN2", target_bir_lowering=False, debug=False, num_devices=NCORES)

    xs_d = nc.dram_tensor("xs", [TOK, E], F32, kind="ExternalInput")
    mt_d = nc.dram_tensor("MT", [DH, DH], F32R, kind="ExternalInput")
    wu_d = nc.dram_tensor("WU", [DH, E], F32R, kind="ExternalInput")
    w1_d = nc.dram_tensor("W1", [E, FF], F32R, kind="ExternalInput")
    w2_d = nc.dram_tensor("W2", [FF, E], F32R, kind="ExternalInput")
    bp_d = nc.dram_tensor("bp", [E], F32R, kind="ExternalInput")
    b1_d = nc.dram_tensor("b1", [FF], F32, kind="ExternalInput")
    b2_d = nc.dram_tensor("b2", [E], F32R, kind="ExternalInput")
    out_d = nc.dram_tensor("out", [TOK, 4, E], F32, kind="ExternalOutput")
    if debug:
        x1_d = nc.dram_tensor("x1_dbg", [TOK, 4, E], F32, kind="ExternalOutput")
        s2_d = nc.dram_tensor("s2_dbg", [TOK, 4, E], F32, kind="ExternalOutput")
        u_d = nc.dram_tensor("u_dbg", [128, 4, E], F32, kind="ExternalOutput")
        z_d = nc.dram_tensor("z_dbg", [128, 512], F32, kind="ExternalOutput")
        w9_d = nc.dram_tensor("w9_dbg", [128, 9], F32, kind="ExternalOutput")

    with tile.TileContext(nc) as tc:
        with (
            tc.tile_pool(name="cons", bufs=1) as cons,
            tc.tile_pool(name="io", bufs=3) as io,
            tc.tile_pool(name="op", bufs=2) as op,
            tc.tile_pool(name="att", bufs=3) as att,
            tc.tile_pool(name="mid", bufs=2) as mid,
            tc.tile_pool(name="x1p", bufs=3) as x1p,
            tc.tile_pool(name="big", bufs=1) as big,
            tc.tile_pool(name="ps", bufs=6, space="PSUM") as ps,
            tc.tile_pool(name="psz", bufs=2, space="PSUM") as psz,
        ):
            ident = cons.tile([128, 128], F32)
            make_identity(nc, ident)
            ones_f = cons.tile([1, 128], F32)
            nc.vector.memset(ones_f, 1.0)
            ones_col = cons.tile([1, 128], F32R)
            nc.vector.tensor_copy(out=ones_col, in_=ones_f)
            ones_f5 = cons.tile([1, 512], F32)
            nc.vector.memset(ones_f5, 1.0)
            ones_row = cons.tile([1, 512], F32R)
            nc.vector.tensor_copy(out=ones_row, in_=ones_f5)

            mt_sb = cons.tile([128, 128], F32R)
            wu_sb = cons.tile([128, E], F32R)
            w1_sb = cons.tile([128, 4, FF], F32R)
            w2_sb = cons.tile([128, 16, E], F32R)
            bp_row = cons.tile([1, E], F32R)
            b2_row = cons.tile([1, E], F32R)
            b1_cols = cons.tile([128, 16], F32)
            nc.sync.dma_start(out=mt_sb, in_=mt_d.ap())
            nc.sync.dma_start(out=wu_sb, in_=wu_d.ap())
            nc.sync.dma_start(out=w1_sb, in_=w1_d.ap().rearrange("(kc kp) f -> kp kc f", kp=128))
            nc.sync.dma_start(out=w2_sb, in_=w2_d.ap().rearrange("(kc kp) e -> kp kc e", kp=128))
            nc.sync.dma_start(out=bp_row, in_=bp_d.ap()[None, :])
            nc.sync.dma_start(out=b2_row, in_=b2_d.ap()[None, :])
            nc.sync.dma_start(out=b1_cols, in_=b1_d.ap().rearrange("(kc kp) -> kp kc", kp=128))

            def stage_a(it):
                t0 = it * 128
                xt = io.tile([128, E], F32, tag="xt")
                nc.sync.dma_start(out=xt, in_=xs_d.ap()[t0:t0 + 128, :])

                # x^T head-chunks (lhsT for z/u matmuls), rounded to f32r
                xT_ps = ps.tile([128, 512], F32, tag="ps")
                for j in range(4):
                    nc.tensor.transpose(xT_ps[:, j * 128:(j + 1) * 128],
                                        xt[:, j * 128:(j + 1) * 128], ident)
                xT = mid.tile([128, 512], F32R, tag="xT")
                nc.scalar.copy(out=xT, in_=xT_ps)

                # z_j = xh_j @ G^T   [128 tok, 4*128]
                z_ps = psz.tile([128, 512], F32, tag="psz")
                for j in range(4):
                    nc.tensor.matmul(z_ps[:, j * 128:(j + 1) * 128],
                                     xT[:, j * 128:(j + 1) * 128], mt_sb,
                                     start=True, stop=True)

                # u_j = xh_j @ (Wv Wp) + bp   [128 tok, 4, 512]
                u_sb = mid.tile([128, 4, E], F32, tag="u")
                for j in range(4):
                    u_ps = ps.tile([128, 512], F32, tag="ps")
                    nc.tensor.matmul(u_ps, ones_col, bp_row, start=True, stop=False)
                    nc.tensor.matmul(u_ps, xT[:, j * 128:(j + 1) * 128], wu_sb,
                                     start=False, stop=True)
                    nc.scalar.copy(out=u_sb[:, j, :], in_=u_ps)

                # scores l_ij = xh_i . z_j  (fused multiply+reduce)
                L = att.tile([128, 9], F32, tag="L")
                scr = att.tile([128, 128], F32, tag="scr")
                for col, (i, j) in enumerate(PAIRS):
                    nc.vector.scalar_tensor_tensor(
                        out=scr, in0=xt[:, i * 128:(i + 1) * 128], scalar=1.0,
                        in1=z_ps[:, j * 128:(j + 1) * 128],
                        op0=OP.mult, op1=OP.mult,
                        accum_out=L[:, col:col + 1])

                # softmax per causal row (sizes 2/3/4); row 0 is identity
                Ex = att.tile([128, 9], F32, tag="Ex")
                nc.scalar.activation(out=Ex, in_=L, func=AF.Exp)
                S = att.tile([128, 3], F32, tag="S")
                R = att.tile([128, 3], F32, tag="R")
                for r, (c0, c1) in ROW_COLS.items():
                    nc.vector.reduce_sum(out=S[:, r - 1:r], in_=Ex[:, c0:c1],
                                         axis=mybir.AxisListType.X)
                nc.vector.reciprocal(out=R, in_=S)
                W9 = att.tile([128, 9], F32, tag="W9")
                for r, (c0, c1) in ROW_COLS.items():
                    nc.vector.tensor_scalar_mul(out=W9[:, c0:c1], in0=Ex[:, c0:c1],
                                                scalar1=R[:, r - 1:r])

                # mix + residual: s1_i = x + sum_j wei_ij u_j  (row 0: x + u_0)
                x1 = x1p.tile([128, 4, E], F32, tag="x1")
                nc.vector.tensor_add(out=x1[:, 0, :], in0=xt, in1=u_sb[:, 0, :])
                for i in (1, 2):
                    c0 = ROW_COLS[i][0]
                    nc.vector.scalar_tensor_tensor(
                        out=x1[:, i, :], in0=u_sb[:, 0, :], scalar=W9[:, c0:c0 + 1],
                        in1=xt, op0=OP.mult, op1=OP.add)
                    for j in range(1, i + 1):
                        nc.vector.scalar_tensor_tensor(
                            out=x1[:, i, :], in0=u_sb[:, j, :],
                            scalar=W9[:, c0 + j:c0 + j + 1],
                            in1=x1[:, i, :], op0=OP.mult, op1=OP.add)
                c0 = ROW_COLS[3][0]
                nc.vector.scalar_tensor_tensor(
                    out=x1[:, 3, :], in0=u_sb[:, 0, :], scalar=W9[:, c0:c0 + 1],
                    in1=xt, op0=OP.mult, op1=OP.add)
                for j in range(1, 4):
                    nc.vector.scalar_tensor_tensor(
                        out=x1[:, 3, :], in0=u_sb[:, j, :], scalar=W9[:, c0 + j:c0 + j + 1],
                        in1=x1[:, 3, :], op0=OP.mult, op1=OP.add)

                # LN1 in place: x1 = LN(s1)
                _layernorm(nc, att, x1, x1, "ln1")
                if debug:
                    nc.sync.dma_start(out=x1_d.ap()[t0:t0 + 128], in_=x1)
                    if it == 0:
                        zc = att.tile([128, 512], F32, tag="zc")
                        nc.vector.tensor_copy(out=zc, in_=z_ps)
                        nc.sync.dma_start(out=z_d.ap(), in_=zc)
                        nc.sync.dma_start(out=u_d.ap(), in_=u_sb)
                        nc.sync.dma_start(out=w9_d.ap(), in_=W9)

                return x1

            def stage_b(it, x1):
                t0 = it * 128
                # x1^T (f32r) for the FFN: rows r = i*128 + t
                x1T = big.tile([128, 4, 512], F32R, tag="x1T")
                for fc in range(4):
                    x1T_ps = ps.tile([128, 512], F32, tag="ps")
                    for i in range(4):
                        nc.tensor.transpose(x1T_ps[:, i * 128:(i + 1) * 128],
                                            x1[:, i, fc * 128:(fc + 1) * 128], ident)
                    nc.scalar.copy(out=x1T[:, fc, :], in_=x1T_ps)

                # ff1: h^T = gelu(W1^T x1^T + b1)  (16 out-chunks of 128)
                hT = big.tile([128, 16, 512], F32R, tag="hT")
                for oc in range(16):
                    h_ps = ps.tile([128, 512], F32, tag="ps")
                    for k in range(4):
                        nc.tensor.matmul(h_ps, w1_sb[:, k, oc * 128:(oc + 1) * 128],
                                         x1T[:, k, :], start=(k == 0), stop=(k == 3))
                    nc.scalar.activation(out=hT[:, oc, :], in_=h_ps, func=AF.Gelu,
                                         bias=b1_cols[:, oc:oc + 1], scale=1.0)

                # ff2: h2^T = W2^T h^T + b2  (4 out-chunks of 128)
                h2T = mid.tile([128, 4, 512], F32, tag="h2T")
                for ec in range(4):
                    o_ps = ps.tile([128, 512], F32, tag="ps")
                    nc.tensor.matmul(o_ps, b2_row[0:1, ec * 128:(ec + 1) * 128],
                                     ones_row, start=True, stop=False)
                    for k in range(16):
                        nc.tensor.matmul(o_ps, w2_sb[:, k, ec * 128:(ec + 1) * 128],
                                         hT[:, k, :], start=False, stop=(k == 15))
                    nc.scalar.copy(out=h2T[:, ec, :], in_=o_ps)

                # un-transpose h2^T to token-major, residual add, LN2
                o_sb = op.tile([128, 4, E], F32, tag="o")
                for i in range(4):
                    ht_ps = ps.tile([128, 512], F32, tag="ps")
                    for ec in range(4):
                        nc.tensor.transpose(ht_ps[:, ec * 128:(ec + 1) * 128],
                                            h2T[:, ec, i * 128:(i + 1) * 128], ident)
                    nc.vector.tensor_add(out=o_sb[:, i, :], in0=x1[:, i, :], in1=ht_ps)
                if debug:
                    nc.sync.dma_start(out=s2_d.ap()[t0:t0 + 128], in_=o_sb)
                _layernorm(nc, att, o_sb, o_sb, "ln2")

                nc.sync.dma_start(out=out_d.ap()[t0:t0 + 128, :, :], in_=o_sb)

            # software pipeline: attention for tile n+1 is emitted before FFN of tile n
            x1_prev = stage_a(0)
            for it in range(1, NT):
                x1_next = stage_a(it)
                stage_b(it - 1, x1_prev)
                x1_prev = x1_next
            stage_b(NT - 1, x1_prev)

    nc.compile()
    return nc


_NC_CACHE = None


def _get_nc():
    global _NC_CACHE
    if _NC_CACHE is None:
        _NC_CACHE = build_bass()
    return _NC_CACHE


def _make_in_maps(inputs):
    x = np.ascontiguousarray(inputs["x"], dtype=np.float32)
    Wq = np.asarray(inputs["Wq"], dtype=np.float64)
    Wk = np.asarray(inputs["Wk"], dtype=np.float64)
    Wv = np.asarray(inputs["Wv"], dtype=np.float64)
    Wp = np.asarray(inputs["Wp"], dtype=np.float64)

    MT = _round_f32r((Wk @ Wq.T * (512.0 ** -0.5)).astype(np.float32))
    WU = _round_f32r((Wv @ Wp).astype(np.float32))
    W1 = _round_f32r(np.asarray(inputs["W1"], dtype=np.float32))
    W2 = _round_f32r(np.asarray(inputs["W2"], dtype=np.float32))
    bp = _round_f32r(np.asarray(inputs["bp"], dtype=np.float32))
    b1 = np.ascontiguousarray(inputs["b1"], dtype=np.float32)
    b2 = _round_f32r(np.asarray(inputs["b2"], dtype=np.float32))

    shared = {"MT": MT, "WU": WU, "W1": W1, "W2": W2, "bp": bp, "b1": b1, "b2": b2}
    in_maps = []
    for c in range(NCORES):
        xs = np.ascontiguousarray(x[c * BS:(c + 1) * BS].reshape(TOK, E))
        in_maps.append({"xs": xs, **shared})
    return in_maps


def run(inputs, trace=False, trace_kwargs=None):
    import os
    os.environ.setdefault("NEURON_RT_RESET_CORES", "1")
    nc = _get_nc()
    in_maps = _make_in_maps(inputs)
    try:
        res = bass_utils.run_bass_kernel_spmd(
            nc, in_maps, core_ids=list(range(NCORES)),
            trace=trace, **(trace_kwargs or {}))
    except Exception:
        res = bass_utils.run_bass_kernel_spmd(
            nc, in_maps, core_ids=list(range(NCORES)),
            trace=trace, **(trace_kwargs or {}))
    outs = [r["out"].reshape(BS, T, 4, E) for r in res.results]
    return np.concatenate(outs, axis=0), res


def kernel(**inputs) -> np.ndarray:
    out, _ = run(inputs)
    return out


# revision 19
# speedup vs baseline: 1.0230x; 1.0230x over previous
"""Trainium2 Bass kernel for nn_Block_8031588843661 (dense_transformer).

The reference block is pointwise over (b, t) tokens: the "attention" runs over
the 4 heads *within* each token (4x4 scores, causal over the head index), so a
pure data-parallel shard of B=4096 across 8 cores needs no communication.

Per-token math (token x of 512 = 4 head-chunks xh_j of 128):
    l[i,j] = xh_i . (G xh_j),  G = Wq Wk^T / sqrt(512)     (j <= i, softmax rows)
    y[i,:] = sum_j wei[i,j] * u_j + bp,  u_j = xh_j @ (Wv Wp)
    x1     = LN_eps512(x + y)
    h      = gelu(x1 @ W1 + b1);  h2 = h @ W2 + b2
    out    = LN_eps512(x1 + h2)                            -> [4, 4, 512] per token

Folding Wq Wk^T and Wv Wp on the host removes the q/k/proj matmuls entirely.
On-chip layout: token-major [128 tokens x feat] for scores/softmax/mix/LN,
feature-major (PE-transposed) for the FFN matmuls, which run in float32r
(fp32 with 11-bit mantissa, full PE rate at free-dim >= 256).

Per core: 512 batches = 2048 tokens = 16 tiles of 128 tokens.
"""

import numpy as np

import concourse.bass as bass
import concourse.bacc as bacc
import concourse.tile as tile
import concourse.mybir as mybir
from concourse import bass_utils
from concourse.masks import make_identity

F32 = mybir.dt.float32
F32R = mybir.dt.float32r
AF = mybir.ActivationFunctionType
OP = mybir.AluOpType

B, T, E, H, DH, FF = 4096, 4, 512, 4, 128, 2048
NCORES = 8
BS = B // NCORES              # 512 batches per core
TOK = BS * T                  # 2048 tokens per core
NT = TOK // 128               # 16 token tiles
LN_EPS = 512.0
RSQRT_Y0 = float((512.0 + 1.5) ** -0.5)   # var+eps lands in ~[512, 515]

# causal (i, j<=i) score pairs for rows i >= 1, in logits-column order
PAIRS = [(1, 0), (1, 1), (2, 0), (2, 1), (2, 2), (3, 0), (3, 1), (3, 2), (3, 3)]
ROW_COLS = {1: (0, 2), 2: (2, 5), 3: (5, 9)}   # logit column ranges per row


def _round_f32r(x: np.ndarray) -> np.ndarray:
    """Round fp32 to fp32r (RNE to 11-bit mantissa) — matches HW cast."""
    u = np.ascontiguousarray(x, dtype=np.float32).view(np.uint32)
    r = (u + (0x7FF + ((u >> 12) & 1))) & np.uint32(0xFFFFF000)
    return r.view(np.float32)


def _rsqrt_dve(nc, pool, a, n):
    """rstd = 1/sqrt(a) on DVE only (no ACT table), a ~ 512..600.
    a: [128, n, 1] fp32 AP. Returns [128, n, 1] tile."""
    y = pool.tile([128, n, 1], F32, tag="rs_y")
    t = pool.tile([128, n, 1], F32, tag="rs_t")
    nc.vector.tensor_scalar(out=y, in0=a, scalar1=0.0, scalar2=RSQRT_Y0,
                            op0=OP.mult, op1=OP.add)
    for _ in range(3):
        nc.vector.tensor_mul(out=t, in0=y, in1=y)
        nc.vector.tensor_mul(out=t, in0=t, in1=a)
        nc.vector.tensor_scalar(out=t, in0=t, scalar1=-0.5, scalar2=1.5,
                                op0=OP.mult, op1=OP.add)
        nc.vector.tensor_mul(out=y, in0=y, in1=t)
    return y


def _layernorm(nc, pool, s, out, tag):
    """out = LN_eps512(s) per 512-feat segment. s/out: [128, 4, 512] SBUF tiles."""
    stats = pool.tile([128, 4, 6], F32, tag=f"{tag}_st")
    mv = pool.tile([128, 4, 2], F32, tag=f"{tag}_mv")
    for i in range(4):
        nc.vector.bn_stats(out=stats[:, i, :], in_=s[:, i, :])
        nc.vector.bn_aggr(out=mv[:, i, :], in_=stats[:, i, :])
    a = pool.tile([128, 4, 1], F32, tag=f"{tag}_a")      # var + eps
    nc.vector.tensor_scalar(out=a, in0=mv[:, :, 1:2], scalar1=LN_EPS, scalar2=None,
                            op0=OP.add)
    rstd = _rsqrt_dve(nc, pool, a, 4)
    for i in range(4):
        nc.vector.tensor_scalar(out=out[:, i, :], in0=s[:, i, :],
                                scalar1=mv[:, i, 0:1], scalar2=rstd[:, i, :],
                                op0=OP.subtract, op1=OP.mult)


def build_bass(debug=False):
    nc = bacc.Bacc("TRN2", target_bir_lowering=False, debug=False, num_devices=NCORES)

    xs_d = nc.dram_tensor("xs", [TOK, E], F32, kind="ExternalInput")
    mt_d = nc.dram_tensor("MT", [DH, DH], F32R, kind="ExternalInput")
    wu_d = nc.dram_tensor("WU", [DH, E], F32R, kind="ExternalInput")
    w1_d = nc.dram_tensor("W1", [E, FF], F32R, kind="ExternalInput")
    w2_d = nc.dram_tensor("W2", [FF, E], F32R, kind="ExternalInput")
    bp_d = nc.dram_tensor("bp", [E], F32R, kind="ExternalInput")
    b1_d = nc.dram_tensor("b1", [FF], F32, kind="ExternalInput")
    b2_d = nc.dram_tensor("b2", [E], F32R, kind="ExternalInput")
    out_d = nc.dram_tensor("out", [TOK, 4, E], F32, kind="ExternalOutput")
    if debug:
        x1_d = nc.dram_tensor("x1_dbg", [TOK, 4, E], F32, kind="ExternalOutput")
        s2_d = nc.dram_tensor("s2_dbg", [TOK, 4, E], F32, kind="ExternalOutput")
        u_d = nc.dram_tensor("u_dbg", [128, 4, E], F32, kind="ExternalOutput")
        z_d = nc.dram_tensor("z_dbg", [128, 512], F32, kind="ExternalOutput")
        w9_d = nc.dram_tensor("w9_dbg", [128, 9], F32, kind="ExternalOutput")

    with tile.TileContext(nc) as tc:
        with (
            tc.tile_pool(name="cons", bufs=1) as cons,
            tc.tile_pool(name="io", bufs=3) as io,
            tc.tile_pool(name="op", bufs=2) as op,
            tc.tile_pool(name="att", bufs=3) as att,
            tc.tile_pool(name="mid", bufs=2) as mid,
            tc.tile_pool(name="x1p", bufs=3) as x1p,
            tc.tile_pool(name="big", bufs=1) as big,
            tc.tile_pool(name="ps", bufs=6, space="PSUM") as ps,
            tc.tile_pool(name="psz", bufs=2, space="PSUM") as psz,
        ):
            ident = cons.tile([128, 128], F32)
            make_identity(nc, ident)

            mt_sb = cons.tile([128, 128], F32R)
            wu_sb = cons.tile([128, E], F32R)
            w1_sb = cons.tile([128, 4, FF], F32R)
            w2_sb = cons.tile([128, 16, E], F32R)
            b1_cols = cons.tile([128, 16], F32)
            bp_bc = cons.tile([128, E], F32)
            nc.gpsimd.dma_start(out=bp_bc, in_=bass.AP(
                tensor=bp_d.ap().bitcast(F32).tensor, offset=0,
                ap=[[0, 128], [1, E]]))
            b2_cols = cons.tile([128, 4], F32)
            nc.sync.dma_start(out=b2_cols, in_=b2_d.ap().bitcast(F32).rearrange("(kc kp) -> kp kc", kp=128))
            nc.sync.dma_start(out=mt_sb, in_=mt_d.ap())
            nc.sync.dma_start(out=wu_sb, in_=wu_d.ap())
            nc.sync.dma_start(out=w1_sb, in_=w1_d.ap().rearrange("(kc kp) f -> kp kc f", kp=128))
            nc.sync.dma_start(out=w2_sb, in_=w2_d.ap().rearrange("(kc kp) e -> kp kc e", kp=128))
            nc.sync.dma_start(out=b1_cols, in_=b1_d.ap().rearrange("(kc kp) -> kp kc", kp=128))

            def stage_a(it):
                t0 = it * 128
                xt = io.tile([128, E], F32, tag="xt")
                nc.sync.dma_start(out=xt, in_=xs_d.ap()[t0:t0 + 128, :])

                # x^T head-chunks (lhsT for z/u matmuls), rounded to f32r
                xT_ps = ps.tile([128, 512], F32, tag="ps")
                for j in range(4):
                    nc.tensor.transpose(xT_ps[:, j * 128:(j + 1) * 128],
                                        xt[:, j * 128:(j + 1) * 128], ident)
                xT = mid.tile([128, 512], F32R, tag="xT")
                nc.scalar.copy(out=xT, in_=xT_ps)

                # z_j = xh_j @ G^T   [128 tok, 4*128]
                z_ps = psz.tile([128, 512], F32, tag="psz")
                for j in range(4):
                    nc.tensor.matmul(z_ps[:, j * 128:(j + 1) * 128],
                                     xT[:, j * 128:(j + 1) * 128], mt_sb,
                                     start=True, stop=True)

                # u_j = xh_j @ (Wv Wp) + bp   [128 tok, 4, 512]
                u_sb = mid.tile([128, 4, E], F32, tag="u")
                for j in range(4):
                    u_ps = ps.tile([128, 512], F32, tag="ps")
                    nc.tensor.matmul(u_ps, xT[:, j * 128:(j + 1) * 128], wu_sb,
                                     start=True, stop=True)
                    nc.scalar.copy(out=u_sb[:, j, :], in_=u_ps)

                # scores l_ij = xh_i . z_j  (fused multiply+reduce)
                L = att.tile([128, 9], F32, tag="L")
                scr = att.tile([128, 128], F32, tag="scr")
                for col, (i, j) in enumerate(PAIRS):
                    nc.vector.scalar_tensor_tensor(
                        out=scr, in0=xt[:, i * 128:(i + 1) * 128], scalar=1.0,
                        in1=z_ps[:, j * 128:(j + 1) * 128],
                        op0=OP.mult, op1=OP.mult,
                        accum_out=L[:, col:col + 1])

                # softmax per causal row (sizes 2/3/4); row 0 is identity
                Ex = att.tile([128, 9], F32, tag="Ex")
                nc.scalar.activation(out=Ex, in_=L, func=AF.Exp)
                S = att.tile([128, 3], F32, tag="S")
                R = att.tile([128, 3], F32, tag="R")
                for r, (c0, c1) in ROW_COLS.items():
                    nc.vector.reduce_sum(out=S[:, r - 1:r], in_=Ex[:, c0:c1],
                                         axis=mybir.AxisListType.X)
                nc.vector.reciprocal(out=R, in_=S)
                W9 = att.tile([128, 9], F32, tag="W9")
                for r, (c0, c1) in ROW_COLS.items():
                    nc.vector.tensor_scalar_mul(out=W9[:, c0:c1], in0=Ex[:, c0:c1],
                                                scalar1=R[:, r - 1:r])

                # mix + residual: s1_i = (x + bp) + sum_j wei_ij u'_j  (sum wei = 1)
                xb = att.tile([128, E], F32, tag="xb")
                nc.vector.tensor_add(out=xb, in0=xt, in1=bp_bc)
                x1 = x1p.tile([128, 4, E], F32, tag="x1")
                nc.vector.tensor_add(out=x1[:, 0, :], in0=xb, in1=u_sb[:, 0, :])
                for i in (1, 2):
                    c0 = ROW_COLS[i][0]
                    nc.vector.scalar_tensor_tensor(
                        out=x1[:, i, :], in0=u_sb[:, 0, :], scalar=W9[:, c0:c0 + 1],
                        in1=xb, op0=OP.mult, op1=OP.add)
                    for j in range(1, i + 1):
                        nc.vector.scalar_tensor_tensor(
                            out=x1[:, i, :], in0=u_sb[:, j, :],
                            scalar=W9[:, c0 + j:c0 + j + 1],
                            in1=x1[:, i, :], op0=OP.mult, op1=OP.add)
                c0 = ROW_COLS[3][0]
                nc.vector.scalar_tensor_tensor(
                    out=x1[:, 3, :], in0=u_sb[:, 0, :], scalar=W9[:, c0:c0 + 1],
                    in1=xb, op0=OP.mult, op1=OP.add)
                for j in range(1, 4):
                    nc.vector.scalar_tensor_tensor(
                        out=x1[:, 3, :], in0=u_sb[:, j, :], scalar=W9[:, c0 + j:c0 + j + 1],
                        in1=x1[:, 3, :], op0=OP.mult, op1=OP.add)

                # LN1 in place: x1 = LN(s1)
                _layernorm(nc, att, x1, x1, "ln1")
                if debug:
                    nc.sync.dma_start(out=x1_d.ap()[t0:t0 + 128], in_=x1)
                    if it == 0:
                        zc = att.tile([128, 512], F32, tag="zc")
                        nc.vector.tensor_copy(out=zc, in_=z_ps)
                        nc.sync.dma_start(out=z_d.ap(), in_=zc)
                        nc.sync.dma_start(out=u_d.ap(), in_=u_sb)
                        nc.sync.dma_start(out=w9_d.ap(), in_=W9)

                return x1

            def stage_b(it, x1):
                t0 = it * 128
                # x1^T (f32r) for the FFN: rows r = i*128 + t
                x1T = big.tile([128, 4, 512], F32R, tag="x1T")
                for fc in range(4):
                    x1T_ps = ps.tile([128, 512], F32, tag="ps")
                    for i in range(4):
                        nc.tensor.transpose(x1T_ps[:, i * 128:(i + 1) * 128],
                                            x1[:, i, fc * 128:(fc + 1) * 128], ident)
                    nc.scalar.copy(out=x1T[:, fc, :], in_=x1T_ps)

                # ff1: h^T = gelu(W1^T x1^T + b1)  (16 out-chunks of 128)
                hT = big.tile([128, 16, 512], F32R, tag="hT")
                for oc in range(16):
                    h_ps = ps.tile([128, 512], F32, tag="ps")
                    for k in range(4):
                        nc.tensor.matmul(h_ps, w1_sb[:, k, oc * 128:(oc + 1) * 128],
                                         x1T[:, k, :], start=(k == 0), stop=(k == 3))
                    nc.scalar.activation(out=hT[:, oc, :], in_=h_ps, func=AF.Gelu,
                                         bias=b1_cols[:, oc:oc + 1], scale=1.0)

                # ff2: h2^T = W2^T h^T + b2  (4 out-chunks of 128)
                h2T = mid.tile([128, 4, 512], F32, tag="h2T")
                for ec in range(4):
                    o_ps = ps.tile([128, 512], F32, tag="ps")
                    for k in range(16):
                        nc.tensor.matmul(o_ps, w2_sb[:, k, ec * 128:(ec + 1) * 128],
                                         hT[:, k, :], start=(k == 0), stop=(k == 15))
                    nc.scalar.activation(out=h2T[:, ec, :], in_=o_ps, func=AF.Identity,
                                         bias=b2_cols[:, ec:ec + 1], scale=1.0)

                # un-transpose h2^T to token-major, residual add, LN2
                o_sb = op.tile([128, 4, E], F32, tag="o")
                for i in range(4):
                    ht_ps = ps.tile([128, 512], F32, tag="ps")
                    for ec in range(4):
                        nc.tensor.transpose(ht_ps[:, ec * 128:(ec + 1) * 128],
                                            h2T[:, ec, i * 128:(i + 1) * 128], ident)
                    nc.vector.tensor_add(out=o_sb[:, i, :], in0=x1[:, i, :], in1=ht_ps)
                if debug:
                    nc.sync.dma_start(out=s2_d.ap()[t0:t0 + 128], in_=o_sb)
                _layernorm(nc, att, o_sb, o_sb, "ln2")

                nc.sync.dma_start(out=out_d.ap()[t0:t0 + 128, :, :], in_=o_sb)

            # software pipeline: attention for tile n+1 is emitted before FFN of tile n
            x1_prev = stage_a(0)
            for it in range(1, NT):
                x1_next = stage_a(it)
                stage_b(it - 1, x1_prev)
                x1_prev = x1_next
            stage_b(NT - 1, x1_prev)

    nc.compile()
    return nc


_NC_CACHE = None


def _get_nc():
    global _NC_CACHE
    if _NC_CACHE is None:
        _NC_CACHE = build_bass()
    return _NC_CACHE


def _make_in_maps(inputs):
    x = np.ascontiguousarray(inputs["x"], dtype=np.float32)
    Wq = np.asarray(inputs["Wq"], dtype=np.float64)
    Wk = np.asarray(inputs["Wk"], dtype=np.float64)
    Wv = np.asarray(inputs["Wv"], dtype=np.float64)
    Wp = np.asarray(inputs["Wp"], dtype=np.float64)

    MT = _round_f32r((Wk @ Wq.T * (512.0 ** -0.5)).astype(np.float32))
    WU = _round_f32r((Wv @ Wp).astype(np.float32))
    W1 = _round_f32r(np.asarray(inputs["W1"], dtype=np.float32))
    W2 = _round_f32r(np.asarray(inputs["W2"], dtype=np.float32))
    bp = _round_f32r(np.asarray(inputs["bp"], dtype=np.float32))
    b1 = np.ascontiguousarray(inputs["b1"], dtype=np.float32)
    b2 = _round_f32r(np.asarray(inputs["b2"], dtype=np.float32))

    shared = {"MT": MT, "WU": WU, "W1": W1, "W2": W2, "bp": bp, "b1": b1, "b2": b2}
    in_maps = []
    for c in range(NCORES):
        xs = np.ascontiguousarray(x[c * BS:(c + 1) * BS].reshape(TOK, E))
        in_maps.append({"xs": xs, **shared})
    return in_maps


def run(inputs, trace=False, trace_kwargs=None):
    import os
    os.environ.setdefault("NEURON_RT_RESET_CORES", "1")
    nc = _get_nc()
    in_maps = _make_in_maps(inputs)
    try:
        res = bass_utils.run_bass_kernel_spmd(
            nc, in_maps, core_ids=list(range(NCORES)),
            trace=trace, **(trace_kwargs or {}))
    except Exception:
        res = bass_utils.run_bass_kernel_spmd(
            nc, in_maps, core_ids=list(range(NCORES)),
            trace=trace, **(trace_kwargs or {}))
    outs = [r["out"].reshape(BS, T, 4, E) for r in res.results]
    return np.concatenate(outs, axis=0), res


def kernel(**inputs) -> np.ndarray:
    out, _ = run(inputs)
    return out
